# revision 7
# baseline (speedup 1.0000x reference)
"""Trainium2 Bass kernel: MeanHinAggregator (GNN message passing).

Reference computation (per batch-head element bh):
    z_r  = mean_n(x_neigh_r[bh, n, :]) @ w_neigh_r          (r = 0, 1)
    out  = relu(concat(x_self[bh] @ w_self, (z0 + z1) / 2) + b)

Strategy (pure data parallel over 8 NeuronCores, batch axis sharded):
  * Per core: B_shard=128, H=10 -> 1280 rows, processed in 10 groups of 128.
  * The kernel is memory-bound (44 MB/core fp32).  All streamed tensors are
    downcast to fp16 on the host (rel-err gate is 2e-2; fp16 keeps us at
    ~3e-4): halves DMA bytes, doubles DVE fold throughput (2x_1p mode for
    2-byte packed dtypes), and makes PE matmuls single-pass (1 cycle/row
    vs 4 for fp32).
  * Host packs xn0|xn1 row-wise into one [BH, 2*N*F] tensor; each group is
    one [128, 8192] fp16 tile loaded by TWO half-DMAs, one per HWDGE ring
    (SP gets the xn0 half, ACT the xn1 half) -> queues stay balanced and
    each partition line is a contiguous 8 KiB descriptor (full DMA rate).
  * The mean-over-neighbours reduction: five in-place strided adds on the
    Vector engine fold the 32 neighbour slices to 1 for BOTH relations at
    once (2D access pattern over the packed tile; fp16 2x mode) ->
    ~4.4 us/group, under the ~5.3 us/group DMA.
  * One transposing matmul per operand (lhsT = data, rhs = identity ->
    out[f, bh] = data[bh, f]) puts the three operands in the [f, bh]
    layout the projection needs as lhsT (PE contracts over partitions).
    PSUM -> SBUF copy runs on the Scalar engine (keeps DVE fold-only).
  * Projection: out[bh, d] = lhsT(sumT).T @ w.  The 1/(N*NR) mean scaling
    is folded into host-prescaled fp16 copies of w_neigh_*.  Bias is added
    with a K=1 matmul (lhsT = ones row, rhs = bias row) into PSUM.
  * Final ReLU on the Scalar engine (PSUM -> SBUF, fp32 out), DMA out on
    the ACT ring (SP carries xn0-half + xs, ACT carries xn1-half + out).
"""

import numpy as np

import concourse.bacc as bacc
import concourse.bass as bass
import concourse.tile as tile
from concourse import bass_utils, mybir
from concourse._compat import with_exitstack

B, H, N, F = 1024, 10, 32, 128
HALF = 128
D = 2 * HALF
NR = 2
NCORES = 8
BSH = B // NCORES        # 128 batch rows per core
BH = BSH * H             # 1280 (bh rows per core)
GROUP = 128              # bh rows per group
NF = N * F               # 4096 (one relation's row width)
F32 = mybir.dt.float32
F16 = mybir.dt.float16


@with_exitstack
def _tile_kernel(ctx, tc, outs, ins, ngroups):
    nc = tc.nc
    xn, xs, w_s, w0, w1, bvec, ident_d = ins
    (out_d,) = outs

    const = ctx.enter_context(tc.tile_pool(name="const", bufs=1))
    xpool = ctx.enter_context(tc.tile_pool(name="xp", bufs=5))
    spool = ctx.enter_context(tc.tile_pool(name="sp", bufs=3))
    opool = ctx.enter_context(tc.tile_pool(name="op", bufs=3))
    ppool = ctx.enter_context(tc.tile_pool(name="ps", bufs=2, space="PSUM"))
    pout = ctx.enter_context(tc.tile_pool(name="po", bufs=2, space="PSUM"))

    # Constants ride the ACT ring ahead of the first neighbour tile (131 KiB,
    # ~0.3 us) so the identity is resident before the first transpose.
    ident = const.tile([128, 128], F16, tag="ident")
    nc.scalar.dma_start(ident[:], ident_d[:])
    wS_t = const.tile([128, HALF], F16, tag="wS")
    nc.scalar.dma_start(wS_t[:], w_s[:])
    w0_t = const.tile([128, HALF], F16, tag="w0")
    nc.scalar.dma_start(w0_t[:], w0[:])
    w1_t = const.tile([128, HALF], F16, tag="w1")
    nc.scalar.dma_start(w1_t[:], w1[:])
    b_t = const.tile([1, D], F16, tag="b")
    nc.scalar.dma_start(b_t[:], bvec[:])
    ones_t = const.tile([1, 128], F16, tag="ones")
    nc.vector.memset(ones_t[:], 1.0)

    def issue_loads(g):
        """One packed neighbour tile per group, split across both HWDGE
        rings; x_self rides the SP ring."""
        r = slice(g * GROUP, (g + 1) * GROUP)
        t = xpool.tile([128, 2, NF], F16, tag="t")
        nc.sync.dma_start(t[:, 0, :], xn[r, 0:NF])
        nc.scalar.dma_start(t[:, 1, :], xn[r, NF:2 * NF])
        ts = spool.tile([128, F], F16, tag="ts")
        nc.sync.dma_start(ts[:], xs[r, :])
        return t, ts

    PREFETCH = 3
    pend = [issue_loads(g) for g in range(min(PREFETCH, ngroups))]

    for g in range(ngroups):
        r = slice(g * GROUP, (g + 1) * GROUP)
        t, ts = pend.pop(0)
        if g + PREFETCH < ngroups:
            pend.append(issue_loads(g + PREFETCH))

        # Fold the 32 neighbour slices of BOTH relations to 1 with five
        # in-place strided adds on the Vector engine (fp16 2x_1p).  The 2D
        # access pattern [128, 2, lv*F] covers xn0 and xn1 halves in one
        # instruction.  After folding, t[:, 0:F] = sum_n xn0 and
        # t[:, NF:NF+F] = sum_n xn1.
        for lv in (16, 8, 4, 2, 1):
            nc.vector.tensor_add(t[:, :, 0:lv * F], t[:, :, 0:lv * F],
                                 t[:, :, lv * F:2 * lv * F])

        # One PSUM tile holds all three transposed operands side by side:
        # pacc[:, 0:128] = sum_n x_n0 (as [f, bh]), [:, 128:256] = sum_n x_n1,
        # [:, 256:384] = x_self.  Each is a single transposing matmul.
        pacc = ppool.tile([128, 3 * 128], F32, tag="pacc")
        nc.tensor.matmul(pacc[:, 0:128], t[:, 0, 0:F], ident[:],
                         start=True, stop=True)
        nc.tensor.matmul(pacc[:, 128:256], t[:, 1, 0:F], ident[:],
                         start=True, stop=True)
        nc.tensor.matmul(pacc[:, 256:384], ts[:], ident[:],
                         start=True, stop=True)

        # PSUM -> SBUF on the Scalar engine (keeps DVE free for folding),
        # casting to fp16 for the projection lhsT.
        sacc = spool.tile([128, 3 * 128], F16, tag="sacc")
        nc.scalar.activation(sacc[:], pacc[:],
                             mybir.ActivationFunctionType.Copy)

        # Projection: out[bh, d]; bias broadcast via K=1 matmuls.
        po = pout.tile([128, D], F32, tag="po")
        nc.tensor.matmul(po[:, 0:HALF], sacc[:, 256:384], wS_t[:],
                         start=True, stop=False)
        nc.tensor.matmul(po[:, 0:HALF], ones_t[:], b_t[:, 0:HALF],
                         start=False, stop=True)
        nc.tensor.matmul(po[:, HALF:D], sacc[:, 0:128], w0_t[:],
                         start=True, stop=False)
        nc.tensor.matmul(po[:, HALF:D], sacc[:, 128:256], w1_t[:],
                         start=False, stop=False)
        nc.tensor.matmul(po[:, HALF:D], ones_t[:], b_t[:, HALF:D],
                         start=False, stop=True)

        ob = opool.tile([128, D], F32, tag="ob")
        nc.scalar.activation(ob[:], po[:], mybir.ActivationFunctionType.Relu)
        nc.scalar.dma_start(out_d[r, :], ob[:])


def build_nc(ngroups=BH // GROUP):
    bh = ngroups * GROUP
    nc = bacc.Bacc("TRN2", target_bir_lowering=False, debug=False)
    xn = nc.dram_tensor("xn", [bh, 2 * NF], F16, kind="ExternalInput")
    xs = nc.dram_tensor("xs", [bh, F], F16, kind="ExternalInput")
    w_s = nc.dram_tensor("w_s", [F, HALF], F16, kind="ExternalInput")
    w0 = nc.dram_tensor("w0", [F, HALF], F16, kind="ExternalInput")
    w1 = nc.dram_tensor("w1", [F, HALF], F16, kind="ExternalInput")
    bvec = nc.dram_tensor("bvec", [1, D], F16, kind="ExternalInput")
    ident_d = nc.dram_tensor("ident", [128, 128], F16, kind="ExternalInput")
    out = nc.dram_tensor("out", [bh, D], F32, kind="ExternalOutput")

    ins = [t.ap() for t in (xn, xs, w_s, w0, w1, bvec, ident_d)]
    with tile.TileContext(nc) as tc:
        _tile_kernel(tc, [out.ap()], ins, ngroups)
    nc.compile()
    return nc


def make_in_maps(x_self, x_neigh_0, x_neigh_1, w_self, w_neigh_0, w_neigh_1, b):
    """Shard full inputs into per-core input maps (batch axis, 8 ways)."""
    x_self = np.asarray(x_self, dtype=np.float32).astype(np.float16)
    x_neigh_0 = np.asarray(x_neigh_0, dtype=np.float32).astype(np.float16)
    x_neigh_1 = np.asarray(x_neigh_1, dtype=np.float32).astype(np.float16)
    scale = np.float32(1.0 / (N * NR))
    w_s = np.asarray(w_self, dtype=np.float32).astype(np.float16)
    w0 = (np.asarray(w_neigh_0, dtype=np.float32) * scale).astype(np.float16)
    w1 = (np.asarray(w_neigh_1, dtype=np.float32) * scale).astype(np.float16)
    bvec = np.asarray(b, dtype=np.float32).astype(np.float16).reshape(1, D)
    ident = np.eye(128, dtype=np.float16)

    # Pack both relations row-wise: xn[bh, 0:NF] = xn0, xn[bh, NF:] = xn1.
    xn_full = np.concatenate(
        [x_neigh_0.reshape(B * H, NF), x_neigh_1.reshape(B * H, NF)], axis=1)

    in_maps = []
    for c in range(NCORES):
        bs = slice(c * BSH * H, (c + 1) * BSH * H)
        in_maps.append({
            "xn": np.ascontiguousarray(xn_full[bs]),
            "xs": np.ascontiguousarray(
                x_self[c * BSH:(c + 1) * BSH].reshape(BH, F)),
            "w_s": w_s, "w0": w0, "w1": w1, "bvec": bvec, "ident": ident,
        })
    return in_maps


_NC_CACHE = None


def kernel(x_self, x_neigh_0, x_neigh_1, w_self, w_neigh_0, w_neigh_1, b):
    global _NC_CACHE
    if _NC_CACHE is None:
        _NC_CACHE = build_nc()
    in_maps = make_in_maps(x_self, x_neigh_0, x_neigh_1,
                           w_self, w_neigh_0, w_neigh_1, b)
    res = bass_utils.run_bass_kernel_spmd(
        _NC_CACHE, in_maps, core_ids=list(range(NCORES)))
    out = np.concatenate([r["out"] for r in res.results], axis=0)
    return out.reshape(B, H, D)
